# revision 17
# baseline (speedup 1.0000x reference)
"""Trainium2 Bass kernel for the SE-sweep DAG-RNN (nn_DAG_RNN_se).

Reference semantics (B=32, C=512, H=W=32):
    h[i,j] = relu(x[:,:,i,j] + (h[i-1,j] + h[i,j-1]) @ W_hh)     # [B, C]
    y[i,j] = h[i,j] @ W_yh + bias

Strategy:
  * Data-parallel over batch: 8 cores x 4 batch elements, zero communication.
  * Anti-diagonal wavefront inside a core: diagonal d holds n_d cells; all
    cells of a diagonal are batched into one set of matmuls.
  * State layout is transposed: h^T [C(4x128 partitions), n_d*B_local] so
    W_hh chunks are the stationary matmul operand; N = 4*n_d <= 128.
  * The +x is folded into the PSUM accumulation via an identity matmul
    (start=True writes x, the 4 W-matmuls accumulate), so the scalar engine
    relu reads PSUM directly and the vector engine only builds the
    neighbour-sum h_sum via free-dim shifted adds (cells on a diagonal are
    ordered by row; neighbours on the next diagonal are adjacent slots).
  * fp16 state + weights (1 cyc/row on PE like bf16, ~8x finer mantissa);
    PSUM accumulates fp32.
  * Output transform y = h @ W_yh in [128,512]-wide matmuls per 512-column
    chunk of the hidden buffer, interleaved into PE bubbles as chunks
    complete.

The full (unsharded) numpy contract is `kernel(**inputs)`; the Bass program
is built and compiled once and cached at module level.
"""

import sys

if "/opt/trn_rl_repo" not in sys.path:
    sys.path.insert(0, "/opt/trn_rl_repo")

import numpy as np

import concourse.bass as bass
import concourse.mybir as mybir
import concourse.tile as tile
from concourse import bacc
from concourse import bass_utils

# ---------------------------------------------------------------- constants
B, C, H, W = 32, 512, 32, 32
NCORES = 8
BL = B // NCORES            # local batch per core = 4
ND = H + W - 1              # 63 diagonals
CT = 4                      # channel chunks of 128
P = 128
YCH = 8                     # output column chunks of 512

F32 = mybir.dt.float32
F16 = mybir.dt.float16
ALU = mybir.AluOpType
ACTF = mybir.ActivationFunctionType

N_D = [min(d, H - 1) - max(0, d - (W - 1)) + 1 for d in range(ND)]
IMIN = [max(0, d - (W - 1)) for d in range(ND)]
OFFB = [0] * (ND + 1)
for _d in range(ND):
    OFFB[_d + 1] = OFFB[_d] + N_D[_d] * BL
TOT = OFFB[ND]              # 4096 columns per chunk row

# y output chunks (col0, width). First 512 columns in 128-wide slivers (they
# become ready early, filling PE gaps in the expanding triangle where the
# per-diagonal matmuls are tiny); then 512-wide; the last 512 split in two so
# the forced serial tail after the final diagonal is half as long.
YCHUNKS = ([(i * 128, 128) for i in range(4)]
           + [(i * 512, 512) for i in range(1, 6)]
           + [(3072, 256), (3328, 256)]
           + [(3584, 128), (3712, 128), (3840, 128), (3968, 128)])
YREADY = [min(d for d in range(ND) if OFFB[d + 1] >= c0 + w)
          for (c0, w) in YCHUNKS]
YLAG = 1


def _build_program():
    nc = bacc.Bacc("TRN2", target_bir_lowering=False, debug=False,
                   num_devices=NCORES)

    xs = nc.dram_tensor("xs", [P, CT * TOT], F16, kind="ExternalInput").ap()
    whh = nc.dram_tensor("whh", [C, C], F16, kind="ExternalInput").ap()
    wyh = nc.dram_tensor("wyh", [C, C], F16, kind="ExternalInput").ap()
    ident = nc.dram_tensor("ident", [P, P], F16, kind="ExternalInput").ap()
    biasp = nc.dram_tensor("biasp", [P, CT], F32, kind="ExternalInput").ap()
    y = nc.dram_tensor("y", [C, TOT], F32, kind="ExternalOutput").ap()

    with tile.TileContext(nc) as tc:
        with (
            tc.tile_pool(name="persist", bufs=1) as persist,
            tc.tile_pool(name="hspool", bufs=4) as hspool,
            tc.tile_pool(name="ypool", bufs=4) as ypool,
            tc.tile_pool(name="recps", bufs=6, space="PSUM") as recps,
            tc.tile_pool(name="yps", bufs=2, space="PSUM") as yps,
        ):
            # ---- resident tensors ----
            whh_sb = persist.tile([P, CT * C], F16, name="whh_sb")
            wyh_sb = persist.tile([P, CT * C], F16, name="wyh_sb")
            id_sb = persist.tile([P, P], F16, name="id_sb")
            bias_sb = persist.tile([P, CT], F32, name="bias_sb")
            # hidden state, chunk-major: chunk k occupies cols [k*TOT,(k+1)*TOT)
            hj = persist.tile([P, CT * TOT], F16, name="hj")
            # full input, resident: col q = CT*OFFB[d] + ct*(n_d*BL) + s*BL + b
            xsb = persist.tile([P, CT * TOT], F16, name="xsb")

            # Startup ordering matters: the first diagonals need (in order)
            # a small x prefix, the identity, and W_hh. W_yh/bias are not
            # needed until the first y chunk (~diag 18). Spread across the
            # sync (HWDGE) and gpsimd (SWDGE) queues for parallelism.
            nc.sync.dma_start(xsb[:, 0:1024], xs[:, 0:1024])
            nc.sync.dma_start(id_sb[:], ident[:])
            for k in range(CT):
                nc.sync.dma_start(whh_sb[:, k * C:(k + 1) * C],
                                  whh[k * P:(k + 1) * P, :])
            NXD = 6
            w = (CT * TOT - 1024) // NXD
            for j in range(NXD):
                c0 = 1024 + j * w
                c1 = CT * TOT if j == NXD - 1 else c0 + w
                eng = nc.sync if j % 2 == 0 else nc.gpsimd
                eng.dma_start(xsb[:, c0:c1], xs[:, c0:c1])
            for k in range(CT):
                nc.gpsimd.dma_start(wyh_sb[:, k * C:(k + 1) * C],
                                    wyh[k * P:(k + 1) * P, :])
            nc.gpsimd.dma_start(bias_sb[:], biasp[:])

            def w_slice(wsb, k, ct):
                return wsb[:, k * C + ct * P: k * C + ct * P + P]

            def hjs(k, c0, w):
                """h chunk-k cols [c0, c0+w) as an AP."""
                return hj[:, k * TOT + c0: k * TOT + c0 + w]

            def hj2(kbase, c0, w):
                """strided pair view: chunks kbase,kbase+1, cols [c0,c0+w)."""
                pair = hj[:, kbase * TOT:(kbase + 2) * TOT]
                return pair.rearrange("p (k q) -> p k q", k=2)[:, :, c0:c0 + w]

            y_emitted = [False] * len(YCHUNKS)
            y_pending_tail = []   # (ch, [psy tiles]) with MMs emitted

            def emit_y_mms(ch):
                c0, wd = YCHUNKS[ch]
                tiles = []
                for ct in range(CT):
                    psy = yps.tile([P, 512], F32, tag="psy",
                                   name=f"psy{ch}_{ct}")
                    for k in range(CT):
                        nc.tensor.matmul(
                            psy[:, 0:wd],
                            lhsT=w_slice(wyh_sb, k, ct),
                            rhs=hjs(k, c0, wd),
                            start=(k == 0), stop=(k == CT - 1))
                    tiles.append(psy)
                y_pending_tail.append((ch, tiles))

            def emit_y_tails():
                # bias + copy-out for chunks whose matmuls were emitted at an
                # earlier diagonal: their PSUM is complete by now, so neither
                # engine queues a long semaphore wait (no head-of-line block)
                while y_pending_tail:
                    ch, tiles = y_pending_tail.pop(0)
                    c0, wd = YCHUNKS[ch]
                    for ct in range(CT):
                        psy = tiles[ct]
                        ysb = ypool.tile([P, 512], F32, tag="ysb",
                                         name=f"ysb{ch}_{ct}")
                        if ct % 2 == 0:
                            nc.vector.tensor_scalar_add(
                                ysb[:, 0:wd], psy[:, 0:wd],
                                bias_sb[:, ct:ct + 1])
                        else:
                            nc.scalar.activation(ysb[:, 0:wd], psy[:, 0:wd],
                                                 ACTF.Identity,
                                                 bias=bias_sb[:, ct:ct + 1],
                                                 scale=1.0)
                        nc.sync.dma_start(
                            y[ct * P:(ct + 1) * P, c0:c0 + wd],
                            ysb[:, 0:wd])

            hs_prev = None     # list of 2 pair tiles [P, 2*N]
            for d in range(ND):
                n = N_D[d]
                N = n * BL
                x0 = CT * OFFB[d]
                emit_y_tails()

                if d + 1 < ND:
                    N2 = N_D[d + 1] * BL
                    hs_next = [hspool.tile([P, 2 * N2], F16, tag=f"hsp{pr}",
                                           name=f"hsp{pr}_{d + 1}")
                               for pr in range(2)]
                else:
                    hs_next = None

                if d == 0:
                    # h = relu(x): two pair-strided activations
                    for pr in range(2):
                        xv = xsb[:, x0 + 2 * pr * N: x0 + (2 * pr + 2) * N]
                        xv = xv.rearrange("p (k q) -> p k q", k=2)
                        nc.scalar.activation(hj2(2 * pr, OFFB[d], N), xv,
                                             ACTF.Relu)
                else:
                    psg = [recps.tile([P, 512], F32, tag="ps",
                                      name=f"ps{d}_{g}")
                           for g in range(CT)]
                    # early sub-burst: x (identity) + pair-A h_sum chunks.
                    # pair-B chunks of the previous diagonal finish ~1us
                    # later, so deferring their matmuls decouples the PE
                    # stream from the h_sum production chain.
                    for g in range(CT):
                        nc.tensor.matmul(psg[g][:, 0:N], lhsT=id_sb[:],
                                         rhs=xsb[:, x0 + g * N:
                                                 x0 + (g + 1) * N],
                                         start=True, stop=False)
                        for k in (0, 1) if g % 2 == 0 else (1, 0):
                            nc.tensor.matmul(
                                psg[g][:, 0:N],
                                lhsT=w_slice(whh_sb, k, g),
                                rhs=hs_prev[0][:, k * N:(k + 1) * N],
                                start=False, stop=False)
                    # mid-diagonal: y matmuls fill the PE while the pair-B
                    # h_sum of the previous diagonal is still being produced
                    for ch in range(len(YCHUNKS)):
                        if not y_emitted[ch] and YREADY[ch] + YLAG <= d:
                            y_emitted[ch] = True
                            emit_y_mms(ch)
                            break
                    # late sub-burst: pair-B chunks, stop on the last;
                    # relu per chunk right after its stop, alternating
                    # engines so the two relus of a pair run concurrently
                    for g in range(CT):
                        ks = (2, 3) if g % 2 == 0 else (3, 2)
                        for idx, k in enumerate(ks):
                            nc.tensor.matmul(
                                psg[g][:, 0:N],
                                lhsT=w_slice(whh_sb, k, g),
                                rhs=hs_prev[1][:, (k - 2) * N:
                                               (k - 1) * N],
                                start=False, stop=(idx == 1))
                        if g % 2 == 0:
                            nc.vector.tensor_scalar_max(
                                hjs(g, OFFB[d], N), psg[g][:, 0:N], 0.0)
                        else:
                            nc.scalar.activation(hjs(g, OFFB[d], N),
                                                 psg[g][:, 0:N], ACTF.Relu)

                # h_sum for diag d+1 from h (pair-strided shifted adds)
                if hs_next is not None:
                    for pr in range(2):
                        hs = hs_next[pr]
                        hsv = hs.rearrange("p (k q) -> p k q", k=2)
                        if d + 1 <= W - 1:
                            # expanding: n2 = n+1
                            nc.vector.tensor_scalar_add(
                                hsv[:, :, 0:BL],
                                hj2(2 * pr, OFFB[d], BL), 0.0)
                            nc.vector.tensor_scalar_add(
                                hsv[:, :, n * BL:(n + 1) * BL],
                                hj2(2 * pr, OFFB[d] + (n - 1) * BL, BL), 0.0)
                            if n > 1:
                                nc.vector.scalar_tensor_tensor(
                                    out=hsv[:, :, BL:n * BL],
                                    in0=hj2(2 * pr, OFFB[d], (n - 1) * BL),
                                    scalar=0.0, op0=ALU.bypass, op1=ALU.add,
                                    in1=hj2(2 * pr, OFFB[d] + BL,
                                            (n - 1) * BL))
                        else:
                            # contracting: n2 = n-1; hs[s] = h[s] + h[s+1]
                            nc.vector.scalar_tensor_tensor(
                                out=hsv[:, :, 0:(n - 1) * BL],
                                in0=hj2(2 * pr, OFFB[d], (n - 1) * BL),
                                scalar=0.0, op0=ALU.bypass, op1=ALU.add,
                                in1=hj2(2 * pr, OFFB[d] + BL, (n - 1) * BL))

                hs_prev = hs_next

            for ch in range(len(YCHUNKS)):
                if not y_emitted[ch]:
                    y_emitted[ch] = True
                    emit_y_mms(ch)
                    emit_y_tails()

    nc.compile()
    return nc


_CACHE = {}


def _get_program():
    if "nc" not in _CACHE:
        _CACHE["nc"] = _build_program()
    return _CACHE["nc"]


def _host_indices():
    """Precompute gather indices for host-side pre/post permutation."""
    if "idx" in _CACHE:
        return _CACHE["idx"]
    ct_of = np.empty(CT * TOT, dtype=np.int64)
    cell_of = np.empty(CT * TOT, dtype=np.int64)
    b_of = np.empty(CT * TOT, dtype=np.int64)
    cell_base = 0
    for d in range(ND):
        n = N_D[d]
        q0 = CT * OFFB[d]
        blk = n * BL
        for ct in range(CT):
            qs = q0 + ct * blk
            idx = np.arange(blk)
            ct_of[qs:qs + blk] = ct
            cell_of[qs:qs + blk] = cell_base + idx // BL
            b_of[qs:qs + blk] = idx % BL
        cell_base += n
    ci = np.empty(H * W, dtype=np.int64)
    cj = np.empty(H * W, dtype=np.int64)
    qcell = np.empty((H, W), dtype=np.int64)
    cell_base = 0
    for d in range(ND):
        for s in range(N_D[d]):
            i = IMIN[d] + s
            ci[cell_base] = i
            cj[cell_base] = d - i
            qcell[i, d - i] = OFFB[d] + s * BL
            cell_base += 1
    _CACHE["idx"] = (ct_of, cell_of, b_of, ci, cj, qcell)
    return _CACHE["idx"]


def make_in_maps(x, whh, wyh, b):
    ct_of, cell_of, b_of, ci, cj, qcell = _host_indices()
    whh16 = whh.astype(np.float16)
    wyh16 = wyh.astype(np.float16)
    id16 = np.eye(P, dtype=np.float16)
    biasp = np.ascontiguousarray(b.reshape(CT, P).T.astype(np.float32))
    xg = x[:, :, ci, cj]                             # [B, C, 1024]
    in_maps = []
    for c in range(NCORES):
        arr = xg[c * BL:(c + 1) * BL]                # [BL, C, 1024]
        arr3 = arr.reshape(BL, CT, P, H * W).transpose(2, 1, 3, 0)
        xs_core = np.ascontiguousarray(
            arr3[np.arange(P)[:, None], ct_of[None, :], cell_of[None, :],
                 b_of[None, :]].astype(np.float16))
        in_maps.append({"xs": xs_core, "whh": whh16, "wyh": wyh16,
                        "ident": id16, "biasp": biasp})
    return in_maps


def kernel(input, weight_hh, weight_yh, bias):
    x = np.ascontiguousarray(np.asarray(input, dtype=np.float32))
    whh = np.asarray(weight_hh, dtype=np.float32)
    wyh = np.asarray(weight_yh, dtype=np.float32)
    b = np.asarray(bias, dtype=np.float32)

    nc = _get_program()
    in_maps = make_in_maps(x, whh, wyh, b)
    res = bass_utils.run_bass_kernel_spmd(nc, in_maps,
                                          core_ids=list(range(NCORES)))

    _, _, _, _, _, qcell = _host_indices()
    out = np.empty((B, C, H, W), dtype=np.float32)
    qidx = qcell[None, :, :] + np.arange(BL)[:, None, None]
    for c in range(NCORES):
        ydev = res.results[c]["y"]                   # [512, 4096]
        out[c * BL:(c + 1) * BL] = ydev[:, qidx].transpose(1, 0, 2, 3)
    return out
